# revision 1
# baseline (speedup 1.0000x reference)
"""Trainium2 Bass kernel for single-head attention model.

Reference computation (B=4, S=2048, E=1024, fp32):
    q = query @ Wq + bq;  k = key @ Wk + bk;  v = value @ Wv + bv
    scores = (q @ k^T) / sqrt(E)
    out = softmax(scores, axis=-1) @ v

Sharding: 8 cores; core c handles batch b = c // 2, query-row half
h = c % 2 (1024 q-rows). K/V projections for the full batch are
computed redundantly on both cores of a pair (no collectives).

Device layout strategy (all matmuls run in float32r = TF32-like
single-pass mode, 1 cycle/row at free-dim >= 256):
  - host pre-transposes inputs so contraction dims land on partitions:
      xqT[e, s_q], xkT[e, s_k], xvT[e, s_k]
  - QT[e, s_q]  = Wq^T xqT   (lhsT = Wq natural layout)
  - KT[e, s_k]  = Wk^T xkT
  - V[s_k, e]   = xvT^T Wv   (lhsT = xvT slices)
  - scoresT[s_k, s_q_blk] = KT^T_slices @ QT  (transposed scores!)
  - expT = exp(scoresT / 32)  -- no max subtraction; scores are O(1)
  - out_unnorm[s_q, e] = expT^T @ V   (lhsT = expT slices, no transposes)
  - sums[s_q] via DVE partial-sum chain over s_k tiles + one tiny
    ones-matmul per 128-row group to reduce over partitions
  - out = out_unnorm * (1/sums)  (per-partition scalar), DMA out natural
"""

import sys

sys.path.insert(0, "/opt/trn_rl_repo")

from contextlib import ExitStack

import numpy as np

import concourse.bass as bass
import concourse.mybir as mybir
import concourse.tile as tile
from concourse import bacc, bass_utils

F32R = mybir.dt.float32r
F32 = mybir.dt.float32
AF = mybir.ActivationFunctionType

B, S, E = 4, 2048, 1024
N_CORES = 8
SQ = S // 2          # q rows per core
SK = S               # kv rows per core
BQ = 256             # s_q block width in phase D
NBLK = SQ // BQ      # 4 blocks
EK = E // 128        # 8 contraction tiles over e
MK = SK // 128       # 16 s_k tiles
INV_SCALE = 1.0 / float(np.sqrt(E))

_cached = {}


def _build():
    nc = bacc.Bacc("TRN2", target_bir_lowering=False, debug=False,
                   num_devices=N_CORES)

    xqT = nc.dram_tensor("xqT", [E, SQ], F32R, kind="ExternalInput").ap()
    xkT = nc.dram_tensor("xkT", [E, SK], F32R, kind="ExternalInput").ap()
    xvT = nc.dram_tensor("xvT", [E, SK], F32R, kind="ExternalInput").ap()
    wq = nc.dram_tensor("wq", [E, E], F32R, kind="ExternalInput").ap()
    wk = nc.dram_tensor("wk", [E, E], F32R, kind="ExternalInput").ap()
    wv = nc.dram_tensor("wv", [E, E], F32R, kind="ExternalInput").ap()
    # biases pre-arranged on host: b_sb[p, t] = b[t*128 + p]
    bqh = nc.dram_tensor("bqh", [128, EK], F32, kind="ExternalInput").ap()
    bkh = nc.dram_tensor("bkh", [128, EK], F32, kind="ExternalInput").ap()
    bvh = nc.dram_tensor("bvh", [1, E], F32, kind="ExternalInput").ap()
    ones_in = nc.dram_tensor("ones_in", [128, 2], F32R, kind="ExternalInput").ap()
    out = nc.dram_tensor("out", [SQ, E], F32, kind="ExternalOutput").ap()

    with tile.TileContext(nc) as tc, ExitStack() as top:
        # ---- long-lived pools (live to end of kernel) ----
        consts = top.enter_context(tc.tile_pool(name="consts", bufs=1))
        vpool = top.enter_context(tc.tile_pool(name="vpool", bufs=1))

        ones_t = consts.tile([128, 2], F32R)
        nc.sync.dma_start(ones_t[:], ones_in)
        bq_t = consts.tile([128, EK], F32)
        nc.sync.dma_start(bq_t[:], bqh)
        bk_t = consts.tile([128, EK], F32)
        nc.sync.dma_start(bk_t[:], bkh)

        # V[s_k, e]: 16 tiles of [128, E]
        v_tiles = [vpool.tile([128, E], F32R, tag=f"v{m}", name=f"v{m}") for m in range(MK)]

        # ================= phase A: V = xvT^T @ Wv + bv =================
        # xvT streamed in 512-col (s_k) blocks; V output tiles accumulate
        with tc.tile_pool(name="xvblk", bufs=3) as xvp, \
             tc.tile_pool(name="wvp", bufs=1) as wvp, \
             tc.tile_pool(name="bvp", bufs=1) as bvp, \
             tc.tile_pool(name="psA", bufs=2, space="PSUM") as psA:
            bv_row = bvp.tile([1, E], F32)
            nc.sync.dma_start(bv_row[:], bvh)
            bv_bc = bvp.tile([128, E], F32)
            nc.gpsimd.partition_broadcast(bv_bc[:], bv_row[:])
            wv_tiles = [wvp.tile([128, E], F32R, tag=f"wv{k}", name=f"wv{k}") for k in range(EK)]
            for n in range(E // 512):
                for k in range(EK):
                    nc.sync.dma_start(
                        wv_tiles[k][:, n * 512:(n + 1) * 512],
                        wv[k * 128:(k + 1) * 128, n * 512:(n + 1) * 512])
            for mb in range(SK // 512):
                xv_blk = [xvp.tile([128, 512], F32R, tag=f"xvb{k}", name=f"xvb{mb}_{k}")
                          for k in range(EK)]
                for k in range(EK):
                    nc.sync.dma_start(
                        xv_blk[k][:],
                        xvT[k * 128:(k + 1) * 128, mb * 512:(mb + 1) * 512])
                for n in range(E // 512):
                    pss = [psA.tile([128, 512], F32, tag=f"psA{i}", name=f"psA_{mb}_{n}_{i}")
                           for i in range(4)]
                    for k in range(EK):
                        for i in range(4):
                            nc.tensor.matmul(
                                pss[i][:],
                                xv_blk[k][:, i * 128:(i + 1) * 128],
                                wv_tiles[k][:, n * 512:(n + 1) * 512],
                                start=(k == 0), stop=(k == EK - 1))
                    for i in range(4):
                        m = mb * 4 + i
                        nc.vector.tensor_add(
                            v_tiles[m][:, n * 512:(n + 1) * 512],
                            pss[i][:],
                            bv_bc[:, n * 512:(n + 1) * 512])

        # ================= phase B: KT = Wk^T @ xkT + bk =================
        ktpool = top.enter_context(tc.tile_pool(name="ktpool", bufs=1))
        kt_tiles = [ktpool.tile([128, SK], F32R, tag=f"kt{m}", name=f"kt{m}")
                    for m in range(EK)]
        with tc.tile_pool(name="xkblk", bufs=2) as xkp, \
             tc.tile_pool(name="wkp", bufs=1) as wkp, \
             tc.tile_pool(name="psB", bufs=8, space="PSUM") as psB:
            wk_tiles = [wkp.tile([128, E], F32R, tag=f"wk{k}", name=f"wk{k}")
                        for k in range(EK)]
            # m-sliced DMA order: first (nb=0, m=0) group unblocks after 8 slices
            for m in range(EK):
                for k in range(EK):
                    nc.sync.dma_start(
                        wk_tiles[k][:, m * 128:(m + 1) * 128],
                        wk[k * 128:(k + 1) * 128, m * 128:(m + 1) * 128])
            for nb in range(SK // 512):
                xk_blk = [xkp.tile([128, 512], F32R, tag=f"xkb{k}", name=f"xkb{nb}_{k}")
                          for k in range(EK)]
                for k in range(EK):
                    nc.sync.dma_start(
                        xk_blk[k][:],
                        xkT[k * 128:(k + 1) * 128, nb * 512:(nb + 1) * 512])
                for m in range(EK):
                    ps = psB.tile([128, 512], F32, tag="psB")
                    for k in range(EK):
                        nc.tensor.matmul(
                            ps[:],
                            wk_tiles[k][:, m * 128:(m + 1) * 128],
                            xk_blk[k][:],
                            start=(k == 0), stop=(k == EK - 1))
                    nc.vector.tensor_scalar_add(
                        kt_tiles[m][:, nb * 512:(nb + 1) * 512],
                        ps[:], bk_t[:, m:m + 1])

        # ================= phase C: QT = Wq^T @ xqT + bq =================
        qtpool = top.enter_context(tc.tile_pool(name="qtpool", bufs=1))
        qt_tiles = [qtpool.tile([128, SQ], F32R, tag=f"qt{m}", name=f"qt{m}")
                    for m in range(EK)]
        with tc.tile_pool(name="xqp", bufs=1) as xqp, \
             tc.tile_pool(name="wqblk", bufs=2) as wqp, \
             tc.tile_pool(name="psC", bufs=8, space="PSUM") as psC:
            xq_tiles = [xqp.tile([128, SQ], F32R, tag=f"xq{k}", name=f"xq{k}") for k in range(EK)]
            for c in range(SQ // 512):
                for k in range(EK):
                    nc.sync.dma_start(
                        xq_tiles[k][:, c * 512:(c + 1) * 512],
                        xqT[k * 128:(k + 1) * 128, c * 512:(c + 1) * 512])
            for m in range(EK):
                wq_blk = [wqp.tile([128, 128], F32R, tag=f"wqb{k}", name=f"wqb{m}_{k}")
                          for k in range(EK)]
                for k in range(EK):
                    nc.sync.dma_start(
                        wq_blk[k][:],
                        wq[k * 128:(k + 1) * 128, m * 128:(m + 1) * 128])
                for n in range(SQ // 512):
                    ps = psC.tile([128, 512], F32, tag="psC")
                    for k in range(EK):
                        nc.tensor.matmul(
                            ps[:], wq_blk[k][:],
                            xq_tiles[k][:, n * 512:(n + 1) * 512],
                            start=(k == 0), stop=(k == EK - 1))
                    nc.vector.tensor_scalar_add(
                        qt_tiles[m][:, n * 512:(n + 1) * 512],
                        ps[:], bq_t[:, m:m + 1])

        # ================= phase D: attention, blocked over s_q =========
        with tc.tile_pool(name="expp", bufs=2) as expp, \
             tc.tile_pool(name="partp", bufs=2) as partp, \
             tc.tile_pool(name="outp", bufs=1) as outp, \
             tc.tile_pool(name="sumsp", bufs=2) as sumsp, \
             tc.tile_pool(name="psS", bufs=3, space="PSUM") as psS, \
             tc.tile_pool(name="psO", bufs=1, space="PSUM") as psO, \
             tc.tile_pool(name="psSum", bufs=1, space="PSUM") as psSum:
            for blk in range(NBLK):
                q0 = blk * BQ
                # scoresT[s_k, blk] = KT^T @ QT_blk ; exp -> expT tiles
                exp_tiles = []
                for m in range(MK):
                    ps = psS.tile([128, BQ], F32, tag="psS")
                    for k in range(EK):
                        nc.tensor.matmul(
                            ps[:],
                            kt_tiles[k][:, m * 128:(m + 1) * 128],
                            qt_tiles[k][:, q0:q0 + BQ],
                            start=(k == 0), stop=(k == EK - 1))
                    et = expp.tile([128, BQ], F32R, tag=f"exp{m}")
                    nc.scalar.activation(et[:], ps[:], AF.Exp, scale=INV_SCALE)
                    exp_tiles.append(et)

                # partial sums over s_k tiles (DVE chain), last write f32r
                part = partp.tile([128, BQ], F32, tag="part")
                nc.vector.tensor_add(part[:], exp_tiles[0][:].bitcast(F32),
                                     exp_tiles[1][:].bitcast(F32))
                for m in range(2, MK - 1):
                    nc.vector.tensor_add(part[:], part[:],
                                         exp_tiles[m][:].bitcast(F32))
                part_r = partp.tile([128, BQ], F32R, tag="part_r")
                nc.vector.tensor_add(part_r[:], part[:],
                                     exp_tiles[MK - 1][:].bitcast(F32))

                # out_unnorm[s_q, e] = expT^T @ V ; sums via ones-matmul
                for mi in range(BQ // 128):
                    pssum = psSum.tile([128, 2], F32, tag="pssum")
                    nc.tensor.matmul(
                        pssum[:],
                        part_r[:, mi * 128:(mi + 1) * 128],
                        ones_t[:], start=True, stop=True)
                    recip = sumsp.tile([128, 1], F32, tag="recip")
                    nc.vector.reciprocal(recip[:], pssum[:, 0:1])

                    ot = outp.tile([128, E], F32, tag=f"out{mi}")
                    for n in range(E // 512):
                        pso = psO.tile([128, 512], F32, tag=f"psO{mi}_{n}")
                        for m in range(MK):
                            nc.tensor.matmul(
                                pso[:],
                                exp_tiles[m][:, mi * 128:(mi + 1) * 128],
                                v_tiles[m][:, n * 512:(n + 1) * 512],
                                start=(m == 0), stop=(m == MK - 1))
                        nc.vector.tensor_scalar_mul(
                            ot[:, n * 512:(n + 1) * 512], pso[:], recip[:])
                    nc.sync.dma_start(
                        out[q0 + mi * 128:q0 + (mi + 1) * 128, :], ot[:])

    nc.compile()
    return nc


def _get_nc():
    if "nc" not in _cached:
        _cached["nc"] = _build()
    return _cached["nc"]


def kernel(query, key, value, Wq, bq, Wk, bk, Wv, bv, **kw):
    query = np.ascontiguousarray(np.asarray(query, dtype=np.float32))
    key = np.ascontiguousarray(np.asarray(key, dtype=np.float32))
    value = np.ascontiguousarray(np.asarray(value, dtype=np.float32))
    Wq = np.ascontiguousarray(np.asarray(Wq, dtype=np.float32))
    Wk = np.ascontiguousarray(np.asarray(Wk, dtype=np.float32))
    Wv = np.ascontiguousarray(np.asarray(Wv, dtype=np.float32))
    bq = np.asarray(bq, dtype=np.float32)
    bk = np.asarray(bk, dtype=np.float32)
    bv = np.asarray(bv, dtype=np.float32)

    bq_h = np.ascontiguousarray(bq.reshape(EK, 128).T)
    bk_h = np.ascontiguousarray(bk.reshape(EK, 128).T)
    bv_h = np.ascontiguousarray(bv.reshape(1, E))
    ones_h = np.ones((128, 2), dtype=np.float32)

    keyT = {b: np.ascontiguousarray(key[b].T) for b in range(B)}
    valT = {b: np.ascontiguousarray(value[b].T) for b in range(B)}

    in_maps = []
    for c in range(N_CORES):
        b, h = divmod(c, 2)
        qT = np.ascontiguousarray(query[b, h * SQ:(h + 1) * SQ, :].T)
        in_maps.append({
            "xqT": qT, "xkT": keyT[b], "xvT": valT[b],
            "wq": Wq, "wk": Wk, "wv": Wv,
            "bqh": bq_h, "bkh": bk_h, "bvh": bv_h,
            "ones_in": ones_h,
        })

    nc = _get_nc()
    res = bass_utils.run_bass_kernel_spmd(
        nc, in_maps, core_ids=list(range(N_CORES)), **kw)

    full = np.empty((B, S, E), dtype=np.float32)
    for c in range(N_CORES):
        b, h = divmod(c, 2)
        full[b, h * SQ:(h + 1) * SQ, :] = res.results[c]["out"]
    kernel.last_results = res
    return full

